# revision 25
# baseline (speedup 1.0000x reference)
"""Trainium2 Bass kernel for nn_FSMNSeleNetV3 (FSMN stack + channel maxpool + decoder).

Self-contained: hardcodes all shapes from the problem spec and only imports
numpy + the concourse stack from /opt/trn_rl_repo.

Sharding: pure data parallel over batch. Each of the 8 cores processes 4
batches x 4 channels = 16 independent sequences of T=2048 tokens.

Layout: activations use an even/odd time-split layout, all in bf16 (fp32
PSUM accumulation). The 128-dim expand stream e is [128 feat, 2049] with
columns 0:1024 = even times, col 1024 = zero guard, 1025:2049 = odd
times. The 64-dim FSMN h stream uses a BLOCKED pair layout (partitions
0:63 = even times, 64:127 = odd times SHIFTED by one pair-column:
partition c+64 column u holds h_od[u-1]).

The odd-phase shift buys a 6-pass FSMN conv (vs 7 for the unshifted pair
layout, and 6*4 slots is the information-theoretic minimum for the 11
taps x 2 phases): pass k in {-4..1} covers tap deltas {2k (ev<-ev),
2k-1 (ev<-od', od<-ev), 2k-2 (od'<-od')}. The shift itself is free: the
shrink's odd-half matmul streams e columns [1024, 2048) = guard +
e_od[0..1022]. The one lost boundary value (h_od[1023]) is restored by a
1-column matmul into a tiny PSUM tile, copied to a fix column of the h
buffer (which doubles as the right halo).

The conv accumulates INTO the shrink's PSUM bank (start=False): the bank
already holds h in fp32, which supplies the conv identity term (o = h +
left + right) at full precision; the conv weights carry raw taps with a
-1 correction at delta=-2 for the odd phase (whose PSUM content is the
shifted h_od[u-1]) and +1 at delta=0. The layer residual is folded into
the PSUM evacuation as a DVE tensor_tensor add. This also frees PSUM
banks: eps 4 + hps 3 + fps 1 = 8 (exactly the budget).

Evacuation split: h (PSUM->SBUF bf16) on ScalarE, expand relu split
ScalarE/VectorE, conv+residual tensor_tensor and channel maxpool on
VectorE (GpSimd cannot access PSUM and fails codegen for tensor ops).

Scheduling: two sequences in flight with a HALF-LAYER OFFSET — per layer
the emission is [expand A, conv B(l-1), shrink A, expand B, conv A(l),
shrink B], so every evacuation latency (~0.7us per PSUM op) hides behind
~2.6us of the other sequence's conv matmuls, and the engine queues
(in-order) receive consumers right after their producers. Pairs are also
pipelined across each other: pair p's final expands interleave with pair
p+1's unit0/layer-0 warmup. Batch outputs (channel maxpool + tiny
decoder) spread across the next pair's layer iterations; the very last
batch pools incrementally and decodes in 512-column chunks so the tail
pipelines into the output DMA. bf16 weights keep fast weight load active;
e/h buffers are static allocations so halo/guard columns are zeroed once.

Measured: ~400us HW exec (from a 435us baseline); PE union occupancy 92%
with ~8.5us of gaps; VectorE 74%, ScalarE 63%.
"""

import sys

sys.path.insert(0, "/opt/trn_rl_repo")
from contextlib import ExitStack

import numpy as np

import concourse.bass as bass  # noqa: F401
import concourse.mybir as mybir
import concourse.tile as tile
from concourse import bacc
from concourse.bass_utils import run_bass_kernel_spmd

F32 = mybir.dt.float32
BF16 = mybir.dt.bfloat16
AF = mybir.ActivationFunctionType
OP = mybir.AluOpType

NCORES = 8
B, T, C, F = 32, 2048, 4, 120
DL, DP, L, LO, RO, S = 128, 64, 5, 10, 1, 5
BPC = B // NCORES  # batches per core
SEQ = BPC * C  # sequences per core
NP_ = T // 2  # pair columns per sequence (1024)
EW = 2 * NP_ + 1  # e width: ev | guard | od  (2049)
HALO = 4  # left halo (k down to -4)
HW_ = HALO + NP_ + 1  # h width: halo | data | fix col  (1029)
NK = 6  # conv passes, k = kk - 4 in [-4 .. +1]
NH = 8  # static h buffers

# packed bf16 weight tensor column offsets
OFF_WE0 = 0
OFF_WEDUP = OFF_WE0 + DL
OFF_WSB = OFF_WEDUP + L * DL
OFF_WCONV = OFF_WSB + L * 2 * DL
OFF_WD = OFF_WCONV + L * NK * 2 * DP
WPK_COLS = OFF_WD + 8


def build_nc():
    nc = bacc.Bacc("TRN2", target_bir_lowering=False, debug=False, num_devices=NCORES)

    xt_d = nc.dram_tensor("xt", [SEQ, F, T], BF16, kind="ExternalInput")
    we0_d = nc.dram_tensor("we0", [F, DL], BF16, kind="ExternalInput")
    wpk_d = nc.dram_tensor("wpk", [DL, WPK_COLS], BF16, kind="ExternalInput")
    wpk32_d = nc.dram_tensor("wpk32", [DL, 8], F32, kind="ExternalInput")
    out_d = nc.dram_tensor("out", [BPC, S, T], F32, kind="ExternalOutput")

    with tile.TileContext(nc) as tc, ExitStack() as ctx:
        wp = ctx.enter_context(tc.tile_pool(name="weights", bufs=1))
        xp = ctx.enter_context(tc.tile_pool(name="x", bufs=5))
        op_ = ctx.enter_context(tc.tile_pool(name="o", bufs=6))
        fp = ctx.enter_context(tc.tile_pool(name="f", bufs=6))
        pp = ctx.enter_context(tc.tile_pool(name="pooled", bufs=2))
        osb = ctx.enter_context(tc.tile_pool(name="osb", bufs=2))
        eps = ctx.enter_context(tc.tile_pool(name="eps", bufs=4, space="PSUM"))
        hps = ctx.enter_context(tc.tile_pool(name="hps", bufs=3, space="PSUM"))
        fps = ctx.enter_context(tc.tile_pool(name="fps", bufs=1, space="PSUM"))

        # --- weights / constants: tiny early DMAs gate unit-0; the big
        # packed DMA is deferred behind the first x loads ---
        we0_tile = wp.tile([F, DL], BF16)
        wpk32_sb = wp.tile([DL, 8], F32)
        wpk_sb = wp.tile([DL, WPK_COLS], BF16)

        def load_wpk():
            nc.sync.dma_start(
                out=wpk_sb[:, OFF_WEDUP:OFF_WCONV], in_=wpk_d[:, OFF_WEDUP:OFF_WCONV]
            )
            for l in range(L):
                c0 = OFF_WCONV + l * NK * 2 * DP
                c1 = OFF_WCONV + (l + 1) * NK * 2 * DP
                nc.sync.dma_start(out=wpk_sb[:, c0:c1], in_=wpk_d[:, c0:c1])
            nc.sync.dma_start(
                out=wpk_sb[:, OFF_WD:WPK_COLS], in_=wpk_d[:, OFF_WD:WPK_COLS]
            )

        we0_sb = we0_tile[:]

        def wedup_at(l, q):
            c = OFF_WEDUP + l * DL
            return wpk_sb[q : q + DP, c : c + DL]

        def wsb_at(l, half):
            c = OFF_WSB + (l * 2 + half) * DL
            return wpk_sb[:, c : c + DL]

        def wconv_at(l, kk):
            c = OFF_WCONV + (l * NK + kk) * 2 * DP
            return wpk_sb[:, c : c + 2 * DP]

        wd_sb = wpk_sb[:, OFF_WD : OFF_WD + S]
        bias_sb = wpk32_sb
        bd_sb = wpk32_sb[0:S, 6:7]

        # static h buffers: left halo zeroed once; data region + fix column
        # rewritten per (seq, layer)
        h_tiles = []
        for i in range(NH):
            t = wp.tile([2 * DP, HW_], BF16, tag=f"h{i}", name=f"h{i}")
            nc.gpsimd.memset(t[:, 0:HALO], 0.0)
            h_tiles.append(t)

        # static e buffers: guard column (1024) zeroed once and never
        # rewritten (expand evacs only touch [0,1024) and [1025,2049))
        NE = 6
        e_tiles = []
        for i in range(NE):
            t = wp.tile([DL, EW], BF16, tag=f"e{i}", name=f"e{i}")
            nc.gpsimd.memset(t[:, NP_ : NP_ + 1], 0.0)
            e_tiles.append(t)
        ectr = [0]

        class Seq:
            def __init__(self, seq):
                self.seq = seq
                self.e = None
                self.o = None
                self.f = None

        hctr = [0]

        def stage_load(st, chunks=1):
            st.x = xp.tile([F, T], BF16, name="x_sb")
            if chunks == 1:
                nc.sync.dma_start(out=st.x[:], in_=xt_d[st.seq][:])
            else:
                # chunks in unit0 window-consumption order (0, 2, 1, 3)
                for i in (0, 2, 1, 3):
                    nc.sync.dma_start(
                        out=st.x[:, i * 512 : (i + 1) * 512],
                        in_=xt_d[st.seq][:, i * 512 : (i + 1) * 512],
                    )

        def e_dst(st, half, w):
            base = half * (NP_ + 1)  # ev at 0, od at 1025
            return st.e[:, base + w * 512 : base + (w + 1) * 512]

        def evac_relu(dst_ap, pe, bias_col, use_dve):
            # relu(psum + bias): alternate between ScalarE ACT and DVE
            # tensor_scalar so PSUM slots recycle fast
            if use_dve:
                nc.vector.tensor_scalar(
                    dst_ap,
                    pe[:],
                    bias_sb[:, bias_col : bias_col + 1],
                    0.0,
                    OP.add,
                    OP.max,
                )
            else:
                nc.scalar.activation(
                    dst_ap,
                    pe[:],
                    AF.Relu,
                    bias=bias_sb[:, bias_col : bias_col + 1],
                    scale=1.0,
                )

        def new_e(st):
            st.e = e_tiles[ectr[0] % NE]
            ectr[0] += 1

        def expand(st, lcol, bias_col, o_prev):
            # o_prev blocked: rows 0:63 = even half, 64:127 = odd half.
            # K=64 row-tiled pairs (duplicated weight halves at PE rows 0
            # and 64) stream concurrently and share the rhs columns.
            new_e(st)
            for w in range(2):
                ws_ = slice(w * 512, (w + 1) * 512)
                pes = []
                for half in range(2):
                    q = half * DP
                    pe = eps.tile([DL, 512], F32, tag="pe", name="pe")
                    nc.tensor.matmul(
                        pe[:],
                        wedup_at(lcol, q),
                        o_prev[q : q + DP, ws_],
                        tile_position=(q, 0),
                    )
                    pes.append(pe)
                for half in range(2):
                    evac_relu(
                        e_dst(st, half, w), pes[half], bias_col, use_dve=(half == 1)
                    )

        def stage_unit0(st):
            new_e(st)
            for w in (0, 2, 1, 3):
                pe = eps.tile([DL, 512], F32, tag="pe", name="pe")
                nc.tensor.matmul(pe[:], we0_sb, st.x[:, w * 512 : (w + 1) * 512])
                evac_relu(
                    st.e[:, (w // 2) * (NP_ + 1) + (w % 2) * 512 :][:, 0:512],
                    pe,
                    0,
                    use_dve=(w >= 2),
                )

        def stage_shrink(st, l):
            # ---- shrink l: blocked zero-padded halves into one bank; the
            # odd half streams e cols [1024, 2048) = guard + od[0..1022],
            # storing h_od shifted by one pair-column ----
            st.h_ps = [
                hps.tile([2 * DP, 512], F32, tag="hp", name=f"hps{w}")
                for w in range(2)
            ]
            for w in range(2):
                nc.tensor.matmul(
                    st.h_ps[w][:],
                    wsb_at(l, 0),
                    st.e[:, w * 512 : w * 512 + 512],
                    start=True,
                    stop=False,
                )
                nc.tensor.matmul(
                    st.h_ps[w][:],
                    wsb_at(l, 1),
                    st.e[:, NP_ + w * 512 : NP_ + w * 512 + 512],
                    start=False,
                    stop=False,
                    skip_group_check=True,
                )
            # boundary fix: h_od[1023] from e od col 1023 (e col 2048)
            st.f_ps = fps.tile([2 * DP, 1], F32, tag="fx", name="fix_ps")
            nc.tensor.matmul(st.f_ps[:], wsb_at(l, 1), st.e[:, EW - 1 : EW])

        def stage_hevac(st):
            h_sb = h_tiles[hctr[0] % NH]
            hctr[0] += 1
            for w in range(2):
                nc.scalar.copy(
                    h_sb[:, HALO + w * 512 : HALO + (w + 1) * 512], st.h_ps[w][:]
                )
            nc.vector.tensor_copy(h_sb[:, HW_ - 1 : HW_], st.f_ps[:])
            st.h_sb = h_sb

        def stage_conv(st, l):
            # ---- FSMN conv: 6 shifted-pair passes accumulating onto the
            # shrink PSUM (identity rides in fp32); evac per window so the
            # w0 residual add overlaps the w1 accumulation ----
            o_new = op_.tile([2 * DP, NP_], BF16, name="o_sb")
            for w in range(2):
                for kk in range(NK):
                    nc.tensor.matmul(
                        st.h_ps[w][:],
                        wconv_at(l, kk),
                        st.h_sb[:, w * 512 + kk : w * 512 + kk + 512],
                        start=False,
                        stop=(kk == NK - 1),
                        skip_group_check=True,
                    )
                ws_ = slice(w * 512, (w + 1) * 512)
                if l == 0:
                    nc.vector.tensor_copy(o_new[:, ws_], st.h_ps[w][:])
                else:
                    nc.vector.tensor_tensor(o_new[:, ws_], st.h_ps[w][:], st.o[:, ws_], OP.add)
            st.o = o_new

        def stage_final(st):
            # final expand writes f in the plain [128, T] ev|od layout
            st.f = fp.tile([DL, T], BF16, name="f_sb")
            for w in range(2):
                ws_ = slice(w * 512, (w + 1) * 512)
                pes = []
                for half in range(2):
                    q = half * DP
                    pe = eps.tile([DL, 512], F32, tag="pe", name="pe")
                    nc.tensor.matmul(
                        pe[:],
                        wedup_at(L - 1, q),
                        st.o[q : q + DP, ws_],
                        tile_position=(q, 0),
                    )
                    pes.append(pe)
                for half in range(2):
                    evac_relu(
                        st.f[:, half * NP_ + w * 512 : half * NP_ + (w + 1) * 512],
                        pes[half],
                        L,
                        use_dve=(half == 1),
                    )

        def decode(b_out, pooled):
            out_sb = osb.tile([S, T], F32, name="out_sb")
            for w in range(T // 512):
                pd = eps.tile([S, 512], F32, tag="pe", name="pd")
                nc.tensor.matmul(pd[:], wd_sb, pooled[:, w * 512 : (w + 1) * 512])
                nc.scalar.activation(
                    out_sb[:, w * 512 : (w + 1) * 512],
                    pd[:],
                    AF.Identity,
                    bias=bd_sb,
                    scale=1.0,
                )
            nc.sync.dma_start(out=out_d[b_out], in_=out_sb[:])


        # ---- software-pipelined pairs of sequences; the batch output
        # (pool + decode) is deferred into the next pair's layer stream so
        # the PE never waits on it. The last batch pools incrementally so
        # only its decode remains after the final expand ----
        f_by_batch = {b: [None] * C for b in range(BPC)}
        pending_out = [None]
        last_pool = [None]
        npairs = SEQ // 2
        carry = [None]
        for pair in range(npairs):
            if pair == 0:
                sA, sB = Seq(0), Seq(1)
                # first x chunk ahead of everything the first matmul does
                # not need; the Sync queue issues DMAs serially (~0.8us each)
                sA.x = xp.tile([F, T], BF16, name="x_sb")
                nc.sync.dma_start(out=sA.x[:, 0:512], in_=xt_d[0][:, 0:512])
                nc.sync.dma_start(out=we0_tile[:], in_=we0_d[:])
                nc.sync.dma_start(out=wpk32_sb[:], in_=wpk32_d[:])
                for i in (2, 1, 3):
                    nc.sync.dma_start(
                        out=sA.x[:, i * 512 : (i + 1) * 512],
                        in_=xt_d[0][:, i * 512 : (i + 1) * 512],
                    )
                stage_load(sB, chunks=4)
                load_wpk()
            else:
                sA, sB = carry[0]
            nxt = (
                (Seq(2 * pair + 2), Seq(2 * pair + 3))
                if pair + 1 < npairs
                else None
            )
            carry[0] = nxt
            for st in (sA, sB):
                stage_unit0(st)
            # ---- half-layer-offset software pipeline: conv(B, l-1) fills
            # the PE between expand(A, l) and shrink(A, l), so every
            # evacuation latency is hidden behind the other sequence's
            # matmul stream. The warmup of pair p (unit0 + layer-0 shrink)
            # is emitted inside pair p-1's finals (cross-pair pipelining) ----
            if pair == 0:
                stage_unit0(sA)
                stage_shrink(sA, 0)
                stage_hevac(sA)
                stage_unit0(sB)
                stage_conv(sA, 0)
                stage_shrink(sB, 0)
                stage_hevac(sB)
            for l in range(1, L):
                expand(sA, l - 1, l, sA.o)
                stage_conv(sB, l - 1)
                stage_shrink(sA, l)
                stage_hevac(sA)
                expand(sB, l - 1, l, sB.o)
                stage_conv(sA, l)
                stage_shrink(sB, l)
                stage_hevac(sB)
                if pending_out[0] is not None:
                    b_out, fs, pooled = pending_out[0]
                    if l == 1:
                        pooled = pp.tile([DL, T], BF16, name="pooled")
                        nc.vector.tensor_max(pooled[:], fs[0][:], fs[1][:])
                        pending_out[0] = (b_out, fs, pooled)
                    elif l < L - 1:
                        nc.vector.tensor_max(pooled[:], pooled[:], fs[l][:])
                    else:
                        decode(b_out, pooled)
                        pending_out[0] = None
                if l == 2 and pair + 1 < npairs:
                    for stn in nxt:
                        stage_load(stn)
                if l == 2 and pair == npairs - 1:
                    # pre-pool the last batch's first two channels
                    fs = f_by_batch[sA.seq // C]
                    lp = pp.tile([DL, T], BF16, tag="poolL", name="last_pool")
                    nc.vector.tensor_max(lp[:], fs[0][:], fs[1][:])
                    last_pool[0] = lp
            stage_final(sA)
            stage_conv(sB, L - 1)
            if last_pool[0] is not None:
                nc.vector.tensor_max(last_pool[0][:], last_pool[0][:], sA.f[:])
            if nxt is not None:
                stage_unit0(nxt[0])
                stage_final(sB)
                stage_shrink(nxt[0], 0)
                stage_hevac(nxt[0])
                stage_unit0(nxt[1])
                stage_conv(nxt[0], 0)
                stage_shrink(nxt[1], 0)
                stage_hevac(nxt[1])
            else:
                stage_final(sB)
            for st in (sA, sB):
                f_by_batch[st.seq // C][st.seq % C] = st.f
            if last_pool[0] is not None:
                # last sequence: chunk max+decode+store so the tail pipelines
                lp = last_pool[0]
                out_sb = osb.tile([S, T], F32, name="out_sb")
                for w in range(4):
                    ws_ = slice(w * 512, (w + 1) * 512)
                    nc.vector.tensor_max(lp[:, ws_], lp[:, ws_], sB.f[:, ws_])
                    pd = eps.tile([S, 512], F32, tag="pe", name="pd")
                    nc.tensor.matmul(pd[:], wd_sb, lp[:, ws_])
                    nc.scalar.activation(
                        out_sb[:, ws_], pd[:], AF.Identity, bias=bd_sb, scale=1.0
                    )
                    nc.sync.dma_start(out=out_d[BPC - 1][:, ws_], in_=out_sb[:, ws_])
            b = sA.seq // C
            if sB.seq % C == C - 1 and last_pool[0] is None:
                pending_out[0] = (b, f_by_batch[b], None)
                f_by_batch[b] = [None] * C

    nc.compile()
    return nc


_NC = None


def get_nc():
    global _NC
    if _NC is None:
        _NC = build_nc()
    return _NC


def prep_in_maps(x, We0, be0, Ws0, wl0, wr0, We, be, Ws, wl, wr, We2, be2, Wd, bd):
    import ml_dtypes

    bf16 = ml_dtypes.bfloat16

    # x [B, T, C, F] -> [B, C, F, T] with even|odd time blocks
    xb = np.ascontiguousarray(x.transpose(0, 2, 3, 1), dtype=np.float32)
    xt = np.concatenate([xb[..., 0::2], xb[..., 1::2]], axis=-1).astype(bf16)

    # expand weights: o blocked -> duplicated halves for row-tiled pairs
    we_list = [We[0], We[1], We[2], We[3], We2]
    wedup = np.stack([np.concatenate([w, w], axis=0) for w in we_list]).astype(bf16)

    # shrink weights, blocked zero-padded columns: e -> h blocked
    ws_list = [Ws0, Ws[0], Ws[1], Ws[2], Ws[3]]
    wsb = np.zeros((L, 2, DL, DL), np.float32)
    for l in range(L):
        wsb[l, 0][:, 0:DP] = ws_list[l]
        wsb[l, 1][:, DP:DL] = ws_list[l]

    biases = np.stack([be0, be[0], be[1], be[2], be[3], be2], axis=1).astype(
        np.float32
    )  # [128, 6]

    wl_full = np.concatenate([wl0[None], wl], axis=0)  # [L, 10, 64]
    wr_full = np.concatenate([wr0[None], wr], axis=0)  # [L, 1, 64]
    taps64 = np.concatenate([wl_full, wr_full], axis=1)  # [L, 11, 64], raw, j = d+9

    def tap(l, d):
        if -9 <= d <= 1:
            return taps64[l, d + 9]
        return np.zeros(DP, np.float32)

    # 6-pass conv weights Wc[l, kk][p_in, p_out]: rows blocked (ev c,
    # od' c+64 holding h_od[u-1]), cols blocked (ev c, od c+64).
    # k = kk - 4: ev<-ev d=2k, ev<-od'/od<-ev d=2k-1, od<-od' d=2k-2.
    # Identity (o = h + ...) rides in the fp32 PSUM content: ev rows give
    # d=0 (wanted), od rows give d=-2 (unwanted, -1 correction), and the
    # true od identity is +1 at d=0 (k=1 slot).
    Wc = np.zeros((L, NK, 2 * DP, 2 * DP), np.float32)
    cc = np.arange(DP)
    for l in range(L):
        for kk in range(NK):
            k = kk - 4
            Wc[l, kk][cc, cc] = tap(l, 2 * k)
            Wc[l, kk][cc + DP, cc] = tap(l, 2 * k - 1)
            Wc[l, kk][cc, cc + DP] = tap(l, 2 * k - 1)
            v = tap(l, 2 * k - 2).copy()
            if 2 * k - 2 == -2:
                v -= 1.0
            if 2 * k - 2 == 0:
                v += 1.0
            Wc[l, kk][cc + DP, cc + DP] = v
    wconv = np.ascontiguousarray(
        Wc.transpose(2, 0, 1, 3).reshape(2 * DP, L * NK * 2 * DP)
    ).astype(bf16)

    wpk = np.zeros((DL, WPK_COLS), bf16)
    wpk[0:F, OFF_WE0 : OFF_WE0 + DL] = We0.astype(bf16)
    wpk[:, OFF_WEDUP : OFF_WEDUP + L * DL] = wedup.transpose(1, 0, 2).reshape(
        DL, L * DL
    )
    wpk[:, OFF_WSB : OFF_WSB + 2 * L * DL] = (
        wsb.reshape(2 * L, DL, DL).transpose(1, 0, 2).reshape(DL, 2 * L * DL)
    ).astype(bf16)
    wpk[:, OFF_WCONV : OFF_WCONV + L * NK * 2 * DP] = wconv
    wpk[:, OFF_WD : OFF_WD + S] = Wd.astype(bf16)

    wpk32 = np.zeros((DL, 8), np.float32)
    wpk32[:, 0 : L + 1] = biases
    wpk32[0:S, 6] = bd

    shared = dict(
        we0=np.ascontiguousarray(We0.astype(bf16)),
        wpk=np.ascontiguousarray(wpk),
        wpk32=wpk32,
    )
    in_maps = []
    for k in range(NCORES):
        xs = xt[k * BPC : (k + 1) * BPC].reshape(SEQ, F, T)
        m = dict(shared)
        m["xt"] = np.ascontiguousarray(xs)
        in_maps.append(m)
    return in_maps


def postprocess(results):
    full = np.concatenate([r["out"] for r in results], axis=0)  # [B, S, T] ev|od
    res = np.empty((B, T, S), np.float32)
    res[:, 0::2, :] = full[:, :, :NP_].transpose(0, 2, 1)
    res[:, 1::2, :] = full[:, :, NP_:].transpose(0, 2, 1)
    return res


def kernel(**inputs):
    nc = get_nc()
    in_maps = prep_in_maps(**inputs)
    res = run_bass_kernel_spmd(nc, in_maps, core_ids=list(range(NCORES)))
    return postprocess(res.results)
